# revision 50
# baseline (speedup 1.0000x reference)
"""Low-rank linear attention (causal, elu+1 feature map) on 8 trn2 cores.

Sharding: core = 2*b + h  (batch b in 0..3, sequence half h in 0..1).
Each core computes out[b, h*2048:(h+1)*2048, :].  Second-half cores
recompute the running K^T.V state over their 2048-token prefix on device
(prefix V contributions are scaled by sel=h so one SPMD program serves
all 8 cores).

Structure (v14, trace-driven; measured 80.3-81.2us vs the 88.1us v5
baseline): the projection phases run warm and near-roofline; the
attention phase always ends up at 1.2 GHz because the PE's HAM
re-throttles at the first lull and never recovers — so the attention
phase is shrunk instead of fighting the clock:
 - PE warm-up: 9 dummy N=512 matmuls right after the preamble so HAM
   latches K=8/8 before the first real matmul.  The prefix x ships as
   fp8-e3m4 (feeds only the h=1 state sum; fp8 lhsT x bf16 weights),
   Wv is prescaled per-core inside wcat, and the attention phase is
   software-pipelined with row-packed contraction-64 matmul pairs.
 - x DRAM layouts are consumption-ordered: prefix chunk-major,
   main group-major; output chunk-major with 1MB group stores.
 - main projections: K-major [q;k] group matmuls (N=512) + token-major
   [kf|v] chunk matmuls (N=128, same wkv weights as the prefix); the
   per-chunk masked score block is also computed here (warm, dense)
   and parked in SBUF, so the attention phase doesn't pay for it at
   half clock.
 - attention phase (cold): per chunk only num/den/state matmuls plus a
   row-PACKED output projection: attn is duplicated into partitions
   0:64 and 64:128 and Wo.T halves are stacked the same way, so the
   two N=512 contraction-64 matmuls run concurrently in disjoint
   row-groups of the PE array.  Snapshot copy on scalar; evictions
   split scalar/vector.
 - elu+1 = max(z+1, min(exp(z), 1)); V carries 1/16 (num/den scale).
 - PSUM: phase A state(1)+p1(2)+pp2(3)+scores(2) = 8; phase B
   state(1)+ndden(3)+op(4x [C,512]) = 8 banks.

Shapes (hardcoded): B=4, S=4096, D=1024, K=64.  L = S/2 = 2048 tokens
per core, processed in 16 chunks of C=128.
"""

import numpy as np

B, S, D, K = 4, 4096, 1024, 64
L = S // 2          # tokens per core (main), also prefix length
C = 128             # chunk (tokens)
G = 512             # token group for K-major projection matmuls
NCHUNK = L // C     # 16
NGRP = L // G       # 4
NDC = D // 128      # 8 contraction chunks
VS = 1.0 / 16.0     # V scale folded into num and den

_cache = {}


def _build_nc():
    import concourse.bacc as bacc
    import concourse.tile as tile
    from concourse import mybir

    f32 = mybir.dt.float32
    bf16 = mybir.dt.bfloat16
    AF = mybir.ActivationFunctionType
    Alu = mybir.AluOpType

    nc = bacc.Bacc()

    f8e3 = mybir.dt.float8e3

    # chunk-major prefix x (fp8-e3m4: feeds only the h=1 state sum,
    # whose error washes out over the 2048-token reduction):
    # [dd, ci*1024 + d*128 + t]
    xtp2 = nc.declare_dram_parameter("xtp2", [128, NCHUNK * 1024], f8e3,
                                     isOutput=False)
    # group-major main x: [dd, g*4096 + d*512 + t]
    xtm2 = nc.declare_dram_parameter("xtm2", [128, NGRP * 4096], bf16,
                                     isOutput=False)
    # wcat cols: [wkvp 8*128 | wkvm 8*128 | wqk 8*128 | mask 128]
    # (wkvp/wkvm carry Wv prescaled by sel/16 resp. 1/16, so the V
    # evictions are plain copies)
    WKVP0, WKVM0, WQK0 = 0, NDC * 128, 2 * NDC * 128
    MSK0 = 3 * NDC * 128
    WCOLS = MSK0 + C
    wcat = nc.declare_dram_parameter("wcat", [128, WCOLS], bf16,
                                     isOutput=False)
    # Wo.T stacked: rows 0:64 = cols 0:512, rows 64:128 = cols 512:1024
    wot2 = nc.declare_dram_parameter("wot2", [128, 512], bf16,
                                     isOutput=False)
    selc = nc.declare_dram_parameter("selc", [C, 1], f32, isOutput=False)
    # chunk-major output: [t, ci*1024 + dcol]
    out2 = nc.declare_dram_parameter("out2", [128, NCHUNK * 1024], bf16,
                                     isOutput=True)

    import contextlib

    with tile.TileContext(nc) as tc, contextlib.ExitStack() as st:
        if True:
            ec = st.enter_context
            consts = ec(tc.tile_pool(name="consts", bufs=1))
            xp_pool = ec(tc.tile_pool(name="xp", bufs=1))
            xg_pool = ec(tc.tile_pool(name="xg", bufs=1))
            ost_pool = ec(tc.tile_pool(name="ost", bufs=1))
            small = ec(tc.tile_pool(name="small", bufs=6))
            vko_pool = ec(tc.tile_pool(name="vko", bufs=NCHUNK + 1))
            pvko_pool = ec(tc.tile_pool(name="pvko", bufs=NCHUNK + 1))
            qk_pool = ec(tc.tile_pool(name="qk", bufs=NCHUNK + 1))
            atn_pool = ec(tc.tile_pool(name="atn", bufs=NCHUNK + 2))
            atx_pool = ec(tc.tile_pool(name="atx", bufs=3))
            ks_pool = ec(tc.tile_pool(name="ks", bufs=4))
            tmp_pool = ec(tc.tile_pool(name="tmp", bufs=4))
            ptmp_pool = ec(tc.tile_pool(name="ptmp", bufs=4))
            nd1_pool = ec(tc.tile_pool(name="nd1s", bufs=NCHUNK + 1))
            dpre_pool = ec(tc.tile_pool(name="dpre", bufs=NCHUNK + 1))
            state_pool = ec(tc.tile_pool(name="state_pool", bufs=1,
                                         space="PSUM"))
            # ---- weights/consts on the scalar HWDGE queue (selc + wkv
            # first: the first prefix matmul + evict need them) ----
            selc_sb = consts.tile([C, 1], f32, tag="selc")
            nc.scalar.dma_start(out=selc_sb, in_=selc[:, :])
            wcat_sb = consts.tile([128, WCOLS], bf16, tag="wcat")
            nc.scalar.dma_start(out=wcat_sb[:, WKVP0:WKVP0 + NDC * 128],
                                in_=wcat[:, WKVP0:WKVP0 + NDC * 128])
            nc.scalar.dma_start(out=wcat_sb[:, WKVM0:WCOLS],
                                in_=wcat[:, WKVM0:WCOLS])
            wot_sb = consts.tile([128, 512], bf16, tag="wot")
            nc.scalar.dma_start(out=wot_sb, in_=wot2[:, :])
            wkvp_sb = [wcat_sb[:, WKVP0 + d * 128:WKVP0 + (d + 1) * 128]
                       for d in range(NDC)]
            wkvm_sb = [wcat_sb[:, WKVM0 + d * 128:WKVM0 + (d + 1) * 128]
                       for d in range(NDC)]
            wqk_sb = [wcat_sb[:, WQK0 + d * 128:WQK0 + (d + 1) * 128]
                      for d in range(NDC)]
            mask_sb = wcat_sb[:, MSK0:MSK0 + C]

            # ---- x transfers on the sync HWDGE queue ----
            xpc = [xp_pool.tile([128, 1024], f8e3, name=f"xpc{ci}",
                                tag=f"xpc{ci}") for ci in range(NCHUNK)]
            xg = [xg_pool.tile([128, 4096], bf16, name=f"xg{g}",
                               tag=f"xg{g}") for g in range(NGRP)]
            for ci in range(NCHUNK):
                nc.sync.dma_start(out=xpc[ci],
                                  in_=xtp2[:, ci * 1024:(ci + 1) * 1024])
            for g in range(NGRP):
                for half in range(2):
                    lo = g * 4096 + half * 2048
                    nc.sync.dma_start(
                        out=xg[g][:, half * 2048:(half + 1) * 2048],
                        in_=xtm2[:, lo:lo + 2048])

            # den-sum column: carries the same 1/16 scale as V's columns
            onec_sb = consts.tile([C, 1], bf16, tag="onec")
            nc.vector.memset(onec_sb, VS)
            # persistent output staging (chunk-major, matches out2)
            ostage = ost_pool.tile([128, NCHUNK * 1024], bf16, tag="ostage")

            # running state [K, K+1]: cols 0:K = S'[k,m] (1/16-scaled),
            # col K = k_sum/16.
            state_ps = state_pool.tile([K, 1 + K], f32)

            # =============== PE warm-up: dummy matmuls on zeros =======
            wz = consts.tile([128, G], bf16, tag="wz")
            nc.vector.memset(wz, 0.0)

            # =============== PREFIX: token-major [K|V], state sum ======
            pvkos = []
            with tc.tile_pool(name="pp_ps", bufs=4, space="PSUM") as pp_pool:
                with tc.tile_pool(name="warm_ps", bufs=1,
                                  space="PSUM") as warm_pool:
                    warm_ps = warm_pool.tile([128, G], f32)
                    for i in range(9):
                        nc.tensor.matmul(warm_ps, wz[:, 0:128], wz,
                                         start=True, stop=True)
                for ci in range(NCHUNK):
                    pp = pp_pool.tile([C, 2 * K], f32, tag="pp")
                    for d in range(NDC):
                        nc.tensor.matmul(pp,
                                         xpc[ci][:, d * 128:(d + 1) * 128],
                                         wkvp_sb[d],
                                         start=(d == 0), stop=(d == NDC - 1))
                    eu = ptmp_pool.tile([C, K], f32, tag="eu")
                    nc.scalar.activation(eu, pp[:, 0:K], AF.Exp)
                    em = ptmp_pool.tile([C, K], f32, tag="em")
                    nc.vector.tensor_scalar_min(em, eu, 1.0)
                    pvko = pvko_pool.tile([C, 2 * K + 1], bf16, tag="pvko")
                    nc.vector.scalar_tensor_tensor(
                        pvko[:, 0:K], pp[:, 0:K], 1.0, em,
                        op0=Alu.add, op1=Alu.max)
                    # V already carries sel/16 (prescaled weights); the
                    # ones column still comes from selc
                    nc.vector.tensor_copy(pvko[:, K:2 * K], pp[:, K:2 * K])
                    nc.gpsimd.tensor_copy(pvko[:, 2 * K:2 * K + 1], selc_sb)
                    pvkos.append(pvko)
                # state updates after all projections: PE never stalls
                for ci in range(NCHUNK):
                    pvko = pvkos[ci]
                    nc.tensor.matmul(state_ps, pvko[:, 0:K],
                                     pvko[:, K:2 * K + 1],
                                     start=(ci == 0), stop=False,
                                     skip_group_check=True)

            # =============== MAIN projections + score blocks ==========
            qTs, vkos, atms = {}, {}, {}
            nd1ss, dpres = {}, {}
            with (
                tc.tile_pool(name="p1_ps", bufs=2, space="PSUM") as p1_pool,
                tc.tile_pool(name="pp2_ps", bufs=3, space="PSUM") as pp2_pool,
                tc.tile_pool(name="sc_ps", bufs=2, space="PSUM") as sc_pool,
            ):
                for g in range(NGRP):
                    p1g = p1_pool.tile([2 * K, G], f32, tag="p1")
                    for d in range(NDC):
                        nc.tensor.matmul(p1g, wqk_sb[d],
                                         xg[g][:, d * 512:(d + 1) * 512],
                                         start=(d == 0), stop=(d == NDC - 1))
                    eg = tmp_pool.tile([2 * K, G], f32, tag="eg", bufs=2)
                    nc.scalar.activation(eg, p1g, AF.Exp)
                    eg2 = tmp_pool.tile([2 * K, G], f32, tag="eg2", bufs=2)
                    nc.vector.tensor_scalar_min(eg2, eg, 1.0)
                    for c4 in range(4):
                        ci = g * 4 + c4
                        sl = slice(c4 * C, (c4 + 1) * C)
                        kT = qk_pool.tile([K, C], bf16, tag="kT")
                        nc.vector.scalar_tensor_tensor(
                            kT, p1g[K:2 * K, sl], 1.0, eg2[K:2 * K, sl],
                            op0=Alu.add, op1=Alu.max)
                        # qT duplicated into both row halves: the high
                        # half feeds the row-packed den matmul later
                        qT2 = qk_pool.tile([2 * K, C], bf16, tag="qT")
                        nc.vector.scalar_tensor_tensor(
                            qT2[0:K, :], p1g[0:K, sl], 1.0, eg2[0:K, sl],
                            op0=Alu.add, op1=Alu.max)
                        nc.gpsimd.tensor_copy(qT2[K:2 * K, :], qT2[0:K, :])
                        qTs[ci] = qT2
                        # token-major [kf|v|1]; wkvm carries Wv/16
                        pp2 = pp2_pool.tile([C, 2 * K], f32, tag="pp2")
                        for d in range(NDC):
                            lo = d * 512 + c4 * 128
                            nc.tensor.matmul(pp2, xg[g][:, lo:lo + 128],
                                             wkvm_sb[d],
                                             start=(d == 0),
                                             stop=(d == NDC - 1))
                        eu2 = ptmp_pool.tile([C, K], f32, tag="eu2")
                        nc.scalar.activation(eu2, pp2[:, 0:K], AF.Exp)
                        em2 = ptmp_pool.tile([C, K], f32, tag="em2")
                        nc.vector.tensor_scalar_min(em2, eu2, 1.0)
                        vko = vko_pool.tile([C, 2 * K + 1], bf16, tag="vko")
                        nc.vector.scalar_tensor_tensor(
                            vko[:, 0:K], pp2[:, 0:K], 1.0, em2,
                            op0=Alu.add, op1=Alu.max)
                        nc.vector.tensor_copy(vko[:, K:2 * K],
                                              pp2[:, K:2 * K])
                        nc.gpsimd.memset(vko[:, 2 * K:2 * K + 1], VS)
                        vkos[ci] = vko
                        # masked score block now, while the PE is warm
                        at = sc_pool.tile([C, C], f32, tag="sc")
                        nc.tensor.matmul(at, kT, qT2[0:K, :], start=True,
                                         stop=True)
                        atm = atn_pool.tile([C, C], bf16, tag="atm")
                        nc.vector.tensor_tensor(atm, at, mask_sb, Alu.mult)
                        # chain-free attention partials also run warm:
                        # nd1 = v'^T A and den1 = A^T 1 share one PSUM
                        # bank and are parked in SBUF (f32, bit-exact);
                        # the attention phase only adds the state terms
                        nb = sc_pool.tile([128, C + 1], f32, tag="sc")
                        nc.tensor.matmul(nb[0:K, 0:C], vko[:, K:2 * K],
                                         atm, start=True, stop=True)
                        nc.tensor.matmul(nb[:, C:C + 1], atm, onec_sb,
                                         start=True, stop=True)
                        nd1s = nd1_pool.tile([K, C], f32, tag="nd1s")
                        nc.scalar.copy(nd1s, nb[0:K, 0:C])
                        dpre = dpre_pool.tile([C, 1], f32, tag="dpre")
                        nc.vector.tensor_copy(dpre, nb[:, C:C + 1])
                        nd1ss[ci], dpres[ci] = nd1s, dpre

            # prefix state snapshot (ks for chunk 0); duplicated into
            # both row halves for the row-packed den matmul
            ks_init = ks_pool.tile([2 * K, 1 + K], bf16, tag="ks")
            nc.scalar.copy(ks_init[0:K, :], state_ps)
            nc.gpsimd.tensor_copy(ks_init[K:2 * K, :], ks_init[0:K, :])

            # =============== PHASE B: attention =======================
            # Software-pipelined: the chain-free start matmuls
            # (nd1 = v'^T A, den1 = A^T 1) for chunk ci+1 are emitted a
            # chunk early, so the only ks-dependent ops (the row-packed
            # nd2/den2 stops) always find their snapshot ready.
            with (
                tc.tile_pool(name="nd_ps", bufs=3, space="PSUM") as nd_pool,
                tc.tile_pool(name="op_ps", bufs=4, space="PSUM") as op_pool,
            ):
                def outproj(ci, attnx, recip):
                    op1 = op_pool.tile([C, 512], f32, tag="op")
                    op2 = op_pool.tile([C, 512], f32, tag="op")
                    # row-packed: the two contraction-64 matmuls occupy
                    # disjoint row-groups of the array and run
                    # concurrently (both fully dep-ready at emission,
                    # so the scheduler keeps them adjacent)
                    nc.tensor.matmul(op1, attnx[0:K, :], wot_sb[0:K, :],
                                     start=True, stop=True)
                    nc.tensor.matmul(op2, attnx[K:2 * K, :],
                                     wot_sb[K:2 * K, :],
                                     start=True, stop=True)
                    lo = ci * 1024
                    nc.scalar.activation(ostage[:, lo:lo + 512], op1,
                                         AF.Copy, scale=recip)
                    nc.vector.tensor_scalar_mul(
                        ostage[:, lo + 512:lo + 1024], op2, recip)
                    if ci == 3:
                        nc.sync.dma_start(out=out2[:, 0:4096],
                                          in_=ostage[:, 0:4096])
                    elif ci == 7:
                        nc.sync.dma_start(out=out2[:, 4096:8192],
                                          in_=ostage[:, 4096:8192])
                    elif ci == 11:
                        nc.sync.dma_start(out=out2[:, 8192:12288],
                                          in_=ostage[:, 8192:12288])
                    elif ci == 14:
                        nc.sync.dma_start(out=out2[:, 12288:15360],
                                          in_=ostage[:, 12288:15360])

                ks_prev = ks_init
                prev = None  # (ci, attnx, recip) pending output projection
                for ci in range(NCHUNK):
                    qT2, vko = qTs[ci], vkos[ci]
                    # ks-dependent state terms only, row-packed into
                    # disjoint row groups; the chain-free partials were
                    # computed warm in phase A
                    nd = nd_pool.tile([K, C], f32, tag="nd")
                    den = nd_pool.tile([C, 1], f32, tag="nd")
                    nc.tensor.matmul(nd, ks_prev[0:K, 0:K], qT2[0:K, :],
                                     start=True, stop=True)
                    nc.tensor.matmul(den, qT2[K:2 * K, :],
                                     ks_prev[K:2 * K, K:K + 1],
                                     start=True, stop=True)
                    # state update + snapshot (snapshot on scalar: the
                    # vector queue stays off the serial chain)
                    nc.tensor.matmul(state_ps, vko[:, 0:K],
                                     vko[:, K:2 * K + 1],
                                     start=False, stop=(ci == NCHUNK - 1),
                                     skip_group_check=True)
                    if ci < NCHUNK - 1:
                        ks_i = ks_pool.tile([2 * K, 1 + K], bf16, tag="ks")
                        nc.scalar.copy(ks_i[0:K, :], state_ps)
                        nc.gpsimd.tensor_copy(ks_i[K:2 * K, :],
                                              ks_i[0:K, :])
                        ks_prev = ks_i
                    dsum = small.tile([C, 1], f32, tag="dsum")
                    nc.vector.tensor_tensor(dsum, den, dpres[ci], Alu.add)
                    recip = small.tile([C, 1], f32, tag="recip")
                    nc.vector.reciprocal(recip, dsum)
                    # attn = state term + warm partial, duplicated into
                    # both row halves for the packed output projection
                    attnx = atx_pool.tile([2 * K, C], bf16, tag="attnx")
                    nc.vector.scalar_tensor_tensor(
                        attnx[0:K, :], nd, 0.0, nd1ss[ci],
                        op0=Alu.add, op1=Alu.add)
                    nc.gpsimd.tensor_copy(attnx[K:2 * K, :], attnx[0:K, :])
                    if prev is not None:
                        outproj(*prev)
                    prev = (ci, attnx, recip)
                outproj(*prev)
                # tail: last chunk's store split across the two HWDGE
                # queues so it doesn't serialize the finish
                nc.sync.dma_start(out=out2[:, 15360:15872],
                                  in_=ostage[:, 15360:15872])
                nc.scalar.dma_start(out=out2[:, 15872:16384],
                                    in_=ostage[:, 15872:16384])

    nc.compile()
    worst = []
    for fn in nc.m.functions:
        for blk in fn.blocks:
            for inst in blk.instructions:
                n = len(inst.sync_info.on_wait) if inst.sync_info else 0
                if n > 1 and type(inst).__name__ == "InstMatmult":
                    worst.append((inst.name, n))
    if worst:
        print(f"WARNING: matmuls with >1 wait after lowering: {worst}")
    return nc


def _prep_inputs(x, Wq, Wk, Wv, Wo):
    import ml_dtypes

    bf16 = ml_dtypes.bfloat16
    wqk = np.concatenate([Wq.T, Wk.T], axis=1)                # [D, 2K]
    mask = np.triu(np.ones((C, C), np.float32))               # keep t <= s

    def make_wcat(sel):
        # [wkvp (Wv*sel/16) | wkvm (Wv/16) | wqk | mask]
        wkvp = np.concatenate([Wk.T, Wv.T * (sel / 16.0)], axis=1)
        wkvm = np.concatenate([Wk.T, Wv.T * VS], axis=1)
        return np.concatenate(
            [wkvp[d * 128:(d + 1) * 128, :] for d in range(NDC)]
            + [wkvm[d * 128:(d + 1) * 128, :] for d in range(NDC)]
            + [wqk[d * 128:(d + 1) * 128, :] for d in range(NDC)]
            + [mask],
            axis=1,
        ).astype(bf16)

    wcats = [make_wcat(0.0), make_wcat(1.0)]
    wotT = Wo.T                                               # [K, D]
    wot2 = np.concatenate([wotT[:, 0:512], wotT[:, 512:1024]],
                          axis=0).astype(bf16)                # [128, 512]
    f8e3 = ml_dtypes.float8_e3m4
    zeros_xp = np.zeros((128, NCHUNK * 1024), dtype=f8e3)
    in_maps = []
    for core in range(8):
        b, h = core // 2, core % 2
        xb = x[b].astype(bf16)                                # [S, D]
        # main: [dd, g*4096 + d*512 + t]
        xm = xb[h * L:(h + 1) * L, :]                         # [2048, 1024]
        xm4 = xm.reshape(NGRP, G, NDC, 128).transpose(3, 0, 2, 1)
        xtm2 = np.ascontiguousarray(xm4.reshape(128, NGRP * 4096))
        # prefix: [dd, ci*1024 + d*128 + t]
        if h:
            xp = x[b][0:L, :].astype(f8e3)
            xp4 = xp.reshape(NCHUNK, C, NDC, 128).transpose(3, 0, 2, 1)
            xtp2 = np.ascontiguousarray(xp4.reshape(128, NCHUNK * 1024))
        else:
            xtp2 = zeros_xp
        m = {
            "xtp2": xtp2,
            "xtm2": xtm2,
            "wcat": wcats[h],
            "wot2": wot2,
            "selc": np.full((C, 1), float(h) / 16.0, np.float32),
        }
        in_maps.append(m)
    return in_maps


def _run(inputs, trace=False):
    from concourse.bass_utils import run_bass_kernel_spmd

    if "nc" not in _cache:
        _cache["nc"] = _build_nc()
    nc = _cache["nc"]
    in_maps = _prep_inputs(
        np.asarray(inputs["x"], np.float32),
        np.asarray(inputs["Wq"], np.float32),
        np.asarray(inputs["Wk"], np.float32),
        np.asarray(inputs["Wv"], np.float32),
        np.asarray(inputs["Wo"], np.float32),
    )
    res = run_bass_kernel_spmd(nc, in_maps, list(range(8)), trace=trace)
    out = np.empty((B, S, D), np.float32)
    for core in range(8):
        b, h = core // 2, core % 2
        o = res.results[core]["out2"].astype(np.float32)
        # [128 t, ci*1024 + dcol] -> [2048, 1024]
        o = o.reshape(128, NCHUNK, 1024).transpose(1, 0, 2).reshape(L, D)
        out[b, h * L:(h + 1) * L, :] = o
    return out, res


def kernel(**inputs) -> np.ndarray:
    out, _ = _run(inputs, trace=False)
    return out


# revision 52
# speedup vs baseline: 1.0444x; 1.0444x over previous
"""Low-rank linear attention (causal, elu+1 feature map) on 8 trn2 cores.

Sharding: core = 2*b + h  (batch b in 0..3, sequence half h in 0..1).
Each core computes out[b, h*2048:(h+1)*2048, :].  Second-half cores
recompute the running K^T.V state over their 2048-token prefix on device
(prefix V contributions are scaled by sel=h so one SPMD program serves
all 8 cores).

Structure (v14, trace-driven; measured 80.3-81.2us vs the 88.1us v5
baseline): the projection phases run warm and near-roofline; the
attention phase always ends up at 1.2 GHz because the PE's HAM
re-throttles at the first lull and never recovers — so the attention
phase is shrunk instead of fighting the clock:
 - PE warm-up: 9 dummy N=512 matmuls right after the preamble so HAM
   latches K=8/8 before the first real matmul.  The prefix x ships as
   fp8-e3m4 (feeds only the h=1 state sum; fp8 lhsT x bf16 weights),
   Wv is prescaled per-core inside wcat, and the attention phase is
   software-pipelined with row-packed contraction-64 matmul pairs.
 - x DRAM layouts are consumption-ordered: prefix chunk-major,
   main group-major; output chunk-major with 1MB group stores.
 - main projections: K-major [q;k] group matmuls (N=512) + token-major
   [kf|v] chunk matmuls (N=128, same wkv weights as the prefix); the
   per-chunk masked score block is also computed here (warm, dense)
   and parked in SBUF, so the attention phase doesn't pay for it at
   half clock.
 - attention phase (cold): per chunk only num/den/state matmuls plus a
   row-PACKED output projection: attn is duplicated into partitions
   0:64 and 64:128 and Wo.T halves are stacked the same way, so the
   two N=512 contraction-64 matmuls run concurrently in disjoint
   row-groups of the PE array.  Snapshot copy on scalar; evictions
   split scalar/vector.
 - elu+1 = max(z+1, min(exp(z), 1)); V carries 1/16 (num/den scale).
 - PSUM: phase A state(1)+p1(2)+pp2(3)+scores(2) = 8; phase B
   state(1)+ndden(3)+op(4x [C,512]) = 8 banks.

Shapes (hardcoded): B=4, S=4096, D=1024, K=64.  L = S/2 = 2048 tokens
per core, processed in 16 chunks of C=128.
"""

import numpy as np

B, S, D, K = 4, 4096, 1024, 64
L = S // 2          # tokens per core (main), also prefix length
C = 128             # chunk (tokens)
G = 512             # token group for K-major projection matmuls
NCHUNK = L // C     # 16
NGRP = L // G       # 4
NDC = D // 128      # 8 contraction chunks
VS = 1.0 / 16.0     # V scale folded into num and den

_cache = {}


def _build_nc():
    import concourse.bacc as bacc
    import concourse.tile as tile
    from concourse import mybir

    f32 = mybir.dt.float32
    bf16 = mybir.dt.bfloat16
    AF = mybir.ActivationFunctionType
    Alu = mybir.AluOpType

    nc = bacc.Bacc()

    f8e3 = mybir.dt.float8e3

    # chunk-major prefix x (fp8-e3m4: feeds only the h=1 state sum,
    # whose error washes out over the 2048-token reduction):
    # [dd, ci*1024 + d*128 + t]
    xtp2 = nc.declare_dram_parameter("xtp2", [128, NCHUNK * 1024], f8e3,
                                     isOutput=False)
    # group-major main x: [dd, g*4096 + d*512 + t]
    xtm2 = nc.declare_dram_parameter("xtm2", [128, NGRP * 4096], bf16,
                                     isOutput=False)
    # wcat cols: [wkvp 8*128 | wkvm 8*128 | wqk 8*128 | mask 128]
    # (wkvp/wkvm carry Wv prescaled by sel/16 resp. 1/16, so the V
    # evictions are plain copies)
    WKVP0, WKVM0, WQK0 = 0, NDC * 128, 2 * NDC * 128
    MSK0 = 3 * NDC * 128
    WCOLS = MSK0 + C
    wcat = nc.declare_dram_parameter("wcat", [128, WCOLS], bf16,
                                     isOutput=False)
    # Wo.T stacked: rows 0:64 = cols 0:512, rows 64:128 = cols 512:1024
    wot2 = nc.declare_dram_parameter("wot2", [128, 512], bf16,
                                     isOutput=False)
    selc = nc.declare_dram_parameter("selc", [C, 1], f32, isOutput=False)
    # chunk-major output: [t, ci*1024 + dcol]
    out2 = nc.declare_dram_parameter("out2", [128, NCHUNK * 1024], bf16,
                                     isOutput=True)

    with tile.TileContext(nc) as tc:
        with (
            tc.tile_pool(name="consts", bufs=1) as consts,
            tc.tile_pool(name="xp", bufs=1) as xp_pool,
            tc.tile_pool(name="xg", bufs=1) as xg_pool,
            tc.tile_pool(name="ost", bufs=1) as ost_pool,
            tc.tile_pool(name="small", bufs=6) as small,
            tc.tile_pool(name="vko", bufs=NCHUNK + 1) as vko_pool,
            tc.tile_pool(name="pvko", bufs=NCHUNK + 1) as pvko_pool,
            tc.tile_pool(name="qk", bufs=NCHUNK + 1) as qk_pool,
            tc.tile_pool(name="atn", bufs=NCHUNK + 2) as atn_pool,
            tc.tile_pool(name="atx", bufs=3) as atx_pool,
            tc.tile_pool(name="ks", bufs=4) as ks_pool,
            tc.tile_pool(name="tmp", bufs=4) as tmp_pool,
            tc.tile_pool(name="ptmp", bufs=4) as ptmp_pool,
            tc.tile_pool(name="state_pool", bufs=1, space="PSUM") as state_pool,
        ):
            # ---- weights/consts on the scalar HWDGE queue (selc + wkv
            # first: the first prefix matmul + evict need them) ----
            selc_sb = consts.tile([C, 1], f32, tag="selc")
            nc.scalar.dma_start(out=selc_sb, in_=selc[:, :])
            wcat_sb = consts.tile([128, WCOLS], bf16, tag="wcat")
            nc.scalar.dma_start(out=wcat_sb[:, WKVP0:WKVP0 + NDC * 128],
                                in_=wcat[:, WKVP0:WKVP0 + NDC * 128])
            nc.scalar.dma_start(out=wcat_sb[:, WKVM0:WCOLS],
                                in_=wcat[:, WKVM0:WCOLS])
            wot_sb = consts.tile([128, 512], bf16, tag="wot")
            nc.scalar.dma_start(out=wot_sb, in_=wot2[:, :])
            wkvp_sb = [wcat_sb[:, WKVP0 + d * 128:WKVP0 + (d + 1) * 128]
                       for d in range(NDC)]
            wkvm_sb = [wcat_sb[:, WKVM0 + d * 128:WKVM0 + (d + 1) * 128]
                       for d in range(NDC)]
            wqk_sb = [wcat_sb[:, WQK0 + d * 128:WQK0 + (d + 1) * 128]
                      for d in range(NDC)]
            mask_sb = wcat_sb[:, MSK0:MSK0 + C]

            # ---- x transfers on the sync HWDGE queue ----
            xpc = [xp_pool.tile([128, 1024], f8e3, name=f"xpc{ci}",
                                tag=f"xpc{ci}") for ci in range(NCHUNK)]
            xg = [xg_pool.tile([128, 4096], bf16, name=f"xg{g}",
                               tag=f"xg{g}") for g in range(NGRP)]
            for ci in range(NCHUNK):
                nc.sync.dma_start(out=xpc[ci],
                                  in_=xtp2[:, ci * 1024:(ci + 1) * 1024])
            for g in range(NGRP):
                for half in range(2):
                    lo = g * 4096 + half * 2048
                    nc.sync.dma_start(
                        out=xg[g][:, half * 2048:(half + 1) * 2048],
                        in_=xtm2[:, lo:lo + 2048])

            # den-sum column: carries the same 1/16 scale as V's columns
            onec_sb = consts.tile([C, 1], bf16, tag="onec")
            nc.vector.memset(onec_sb, VS)
            # persistent output staging (chunk-major, matches out2)
            ostage = ost_pool.tile([128, NCHUNK * 1024], bf16, tag="ostage")

            # running state [K, K+1]: cols 0:K = S'[k,m] (1/16-scaled),
            # col K = k_sum/16.
            state_ps = state_pool.tile([K, 1 + K], f32)

            # =============== PE warm-up: dummy matmuls on zeros =======
            wz = consts.tile([128, G], bf16, tag="wz")
            nc.vector.memset(wz, 0.0)

            # =============== PREFIX: token-major [K|V], state sum ======
            pvkos = []
            with tc.tile_pool(name="pp_ps", bufs=4, space="PSUM") as pp_pool:
                with tc.tile_pool(name="warm_ps", bufs=1,
                                  space="PSUM") as warm_pool:
                    warm_ps = warm_pool.tile([128, G], f32)
                    for i in range(7):
                        nc.tensor.matmul(warm_ps, wz[:, 0:128], wz,
                                         start=True, stop=True)
                for ci in range(NCHUNK):
                    pp = pp_pool.tile([C, 2 * K], f32, tag="pp")
                    for d in range(NDC):
                        nc.tensor.matmul(pp,
                                         xpc[ci][:, d * 128:(d + 1) * 128],
                                         wkvp_sb[d],
                                         start=(d == 0), stop=(d == NDC - 1))
                    eu = ptmp_pool.tile([C, K], f32, tag="eu")
                    nc.scalar.activation(eu, pp[:, 0:K], AF.Exp)
                    em = ptmp_pool.tile([C, K], f32, tag="em")
                    nc.vector.tensor_scalar_min(em, eu, 1.0)
                    pvko = pvko_pool.tile([C, 2 * K + 1], bf16, tag="pvko")
                    nc.vector.scalar_tensor_tensor(
                        pvko[:, 0:K], pp[:, 0:K], 1.0, em,
                        op0=Alu.add, op1=Alu.max)
                    # V already carries sel/16 (prescaled weights); the
                    # ones column still comes from selc
                    nc.vector.tensor_copy(pvko[:, K:2 * K], pp[:, K:2 * K])
                    nc.gpsimd.tensor_copy(pvko[:, 2 * K:2 * K + 1], selc_sb)
                    pvkos.append(pvko)
                # state updates after all projections: PE never stalls
                for ci in range(NCHUNK):
                    pvko = pvkos[ci]
                    nc.tensor.matmul(state_ps, pvko[:, 0:K],
                                     pvko[:, K:2 * K + 1],
                                     start=(ci == 0), stop=False,
                                     skip_group_check=True)

            # =============== MAIN projections + score blocks ==========
            qTs, vkos, atms = {}, {}, {}
            with (
                tc.tile_pool(name="p1_ps", bufs=2, space="PSUM") as p1_pool,
                tc.tile_pool(name="pp2_ps", bufs=3, space="PSUM") as pp2_pool,
                tc.tile_pool(name="sc_ps", bufs=2, space="PSUM") as sc_pool,
            ):
                for g in range(NGRP):
                    p1g = p1_pool.tile([2 * K, G], f32, tag="p1")
                    for d in range(NDC):
                        nc.tensor.matmul(p1g, wqk_sb[d],
                                         xg[g][:, d * 512:(d + 1) * 512],
                                         start=(d == 0), stop=(d == NDC - 1))
                    eg = tmp_pool.tile([2 * K, G], f32, tag="eg", bufs=2)
                    nc.scalar.activation(eg, p1g, AF.Exp)
                    eg2 = tmp_pool.tile([2 * K, G], f32, tag="eg2", bufs=2)
                    nc.vector.tensor_scalar_min(eg2, eg, 1.0)
                    for c4 in range(4):
                        ci = g * 4 + c4
                        sl = slice(c4 * C, (c4 + 1) * C)
                        kT = qk_pool.tile([K, C], bf16, tag="kT")
                        nc.vector.scalar_tensor_tensor(
                            kT, p1g[K:2 * K, sl], 1.0, eg2[K:2 * K, sl],
                            op0=Alu.add, op1=Alu.max)
                        # qT duplicated into both row halves: the high
                        # half feeds the row-packed den matmul later
                        qT2 = qk_pool.tile([2 * K, C], bf16, tag="qT")
                        nc.vector.scalar_tensor_tensor(
                            qT2[0:K, :], p1g[0:K, sl], 1.0, eg2[0:K, sl],
                            op0=Alu.add, op1=Alu.max)
                        nc.gpsimd.tensor_copy(qT2[K:2 * K, :], qT2[0:K, :])
                        qTs[ci] = qT2
                        # token-major [kf|v|1]; wkvm carries Wv/16
                        pp2 = pp2_pool.tile([C, 2 * K], f32, tag="pp2")
                        for d in range(NDC):
                            lo = d * 512 + c4 * 128
                            nc.tensor.matmul(pp2, xg[g][:, lo:lo + 128],
                                             wkvm_sb[d],
                                             start=(d == 0),
                                             stop=(d == NDC - 1))
                        eu2 = ptmp_pool.tile([C, K], f32, tag="eu2")
                        nc.scalar.activation(eu2, pp2[:, 0:K], AF.Exp)
                        em2 = ptmp_pool.tile([C, K], f32, tag="em2")
                        nc.vector.tensor_scalar_min(em2, eu2, 1.0)
                        vko = vko_pool.tile([C, 2 * K + 1], bf16, tag="vko")
                        nc.vector.scalar_tensor_tensor(
                            vko[:, 0:K], pp2[:, 0:K], 1.0, em2,
                            op0=Alu.add, op1=Alu.max)
                        nc.vector.tensor_copy(vko[:, K:2 * K],
                                              pp2[:, K:2 * K])
                        nc.gpsimd.memset(vko[:, 2 * K:2 * K + 1], VS)
                        vkos[ci] = vko
                        # masked score block now, while the PE is warm
                        at = sc_pool.tile([C, C], f32, tag="sc")
                        nc.tensor.matmul(at, kT, qT2[0:K, :], start=True,
                                         stop=True)
                        atm = atn_pool.tile([C, C], bf16, tag="atm")
                        nc.vector.tensor_tensor(atm, at, mask_sb, Alu.mult)
                        atms[ci] = atm

            # prefix state snapshot (ks for chunk 0); duplicated into
            # both row halves for the row-packed den matmul
            ks_init = ks_pool.tile([2 * K, 1 + K], bf16, tag="ks")
            nc.scalar.copy(ks_init[0:K, :], state_ps)
            nc.gpsimd.tensor_copy(ks_init[K:2 * K, :], ks_init[0:K, :])

            # =============== PHASE B: attention =======================
            # Software-pipelined: the chain-free start matmuls
            # (nd1 = v'^T A, den1 = A^T 1) for chunk ci+1 are emitted a
            # chunk early, so the only ks-dependent ops (the row-packed
            # nd2/den2 stops) always find their snapshot ready.
            with (
                tc.tile_pool(name="nd_ps", bufs=3, space="PSUM") as nd_pool,
                tc.tile_pool(name="op_ps", bufs=4, space="PSUM") as op_pool,
            ):
                def outproj(ci, attnx, recip):
                    op1 = op_pool.tile([C, 512], f32, tag="op")
                    op2 = op_pool.tile([C, 512], f32, tag="op")
                    # row-packed: the two contraction-64 matmuls occupy
                    # disjoint row-groups of the array and run
                    # concurrently (both fully dep-ready at emission,
                    # so the scheduler keeps them adjacent)
                    nc.tensor.matmul(op1, attnx[0:K, :], wot_sb[0:K, :],
                                     start=True, stop=True)
                    nc.tensor.matmul(op2, attnx[K:2 * K, :],
                                     wot_sb[K:2 * K, :],
                                     start=True, stop=True)
                    lo = ci * 1024
                    nc.scalar.activation(ostage[:, lo:lo + 512], op1,
                                         AF.Copy, scale=recip)
                    nc.vector.tensor_scalar_mul(
                        ostage[:, lo + 512:lo + 1024], op2, recip)
                    if ci == 3:
                        nc.sync.dma_start(out=out2[:, 0:4096],
                                          in_=ostage[:, 0:4096])
                    elif ci == 7:
                        nc.sync.dma_start(out=out2[:, 4096:8192],
                                          in_=ostage[:, 4096:8192])
                    elif ci == 11:
                        nc.sync.dma_start(out=out2[:, 8192:12288],
                                          in_=ostage[:, 8192:12288])
                    elif ci == 14:
                        nc.sync.dma_start(out=out2[:, 12288:15360],
                                          in_=ostage[:, 12288:15360])

                nd_open = {}

                def emit_nd1(ci):
                    """chain-free accumulation starts for chunk ci."""
                    nd = nd_pool.tile([K, C], f32, tag="nd")
                    den = nd_pool.tile([C, 1], f32, tag="nd")
                    nc.tensor.matmul(nd, vkos[ci][:, K:2 * K], atms[ci],
                                     start=True, stop=False)
                    nc.tensor.matmul(den, atms[ci], onec_sb, start=True,
                                     stop=False)
                    nd_open[ci] = (nd, den)

                emit_nd1(0)
                ks_prev = ks_init
                prev = None  # (ci, attnx, recip) pending output projection
                for ci in range(NCHUNK):
                    qT2, vko = qTs[ci], vkos[ci]
                    nd, den = nd_open.pop(ci)
                    # ks-dependent stops, row-packed into disjoint row
                    # groups (snapshot is a full chunk old: no PE stall)
                    nc.tensor.matmul(nd, ks_prev[0:K, 0:K], qT2[0:K, :],
                                     start=False, stop=True)
                    nc.tensor.matmul(den, qT2[K:2 * K, :],
                                     ks_prev[K:2 * K, K:K + 1],
                                     start=False, stop=True)
                    # state update + snapshot (snapshot on scalar: the
                    # vector queue stays off the serial chain)
                    nc.tensor.matmul(state_ps, vko[:, 0:K],
                                     vko[:, K:2 * K + 1],
                                     start=False, stop=(ci == NCHUNK - 1),
                                     skip_group_check=True)
                    if ci < NCHUNK - 1:
                        ks_i = ks_pool.tile([2 * K, 1 + K], bf16, tag="ks")
                        nc.scalar.copy(ks_i[0:K, :], state_ps)
                        nc.gpsimd.tensor_copy(ks_i[K:2 * K, :],
                                              ks_i[0:K, :])
                        ks_prev = ks_i
                    recip = small.tile([C, 1], f32, tag="recip")
                    nc.vector.reciprocal(recip, den)
                    # attn duplicated into both row halves for the
                    # packed output projection
                    attnx = atx_pool.tile([2 * K, C], bf16, tag="attnx")
                    nc.vector.tensor_copy(attnx[0:K, :], nd)
                    nc.gpsimd.tensor_copy(attnx[K:2 * K, :], attnx[0:K, :])
                    if ci + 1 < NCHUNK:
                        emit_nd1(ci + 1)
                    if prev is not None:
                        outproj(*prev)
                    prev = (ci, attnx, recip)
                outproj(*prev)
                # tail: last chunk's store split across the two HWDGE
                # queues so it doesn't serialize the finish
                nc.sync.dma_start(out=out2[:, 15360:15872],
                                  in_=ostage[:, 15360:15872])
                nc.scalar.dma_start(out=out2[:, 15872:16384],
                                    in_=ostage[:, 15872:16384])

    nc.compile()
    worst = []
    for fn in nc.m.functions:
        for blk in fn.blocks:
            for inst in blk.instructions:
                n = len(inst.sync_info.on_wait) if inst.sync_info else 0
                if n > 1 and type(inst).__name__ == "InstMatmult":
                    worst.append((inst.name, n))
    if worst:
        print(f"WARNING: matmuls with >1 wait after lowering: {worst}")
    return nc


def _prep_inputs(x, Wq, Wk, Wv, Wo):
    import ml_dtypes

    bf16 = ml_dtypes.bfloat16
    wqk = np.concatenate([Wq.T, Wk.T], axis=1)                # [D, 2K]
    mask = np.triu(np.ones((C, C), np.float32))               # keep t <= s

    def make_wcat(sel):
        # [wkvp (Wv*sel/16) | wkvm (Wv/16) | wqk | mask]
        wkvp = np.concatenate([Wk.T, Wv.T * (sel / 16.0)], axis=1)
        wkvm = np.concatenate([Wk.T, Wv.T * VS], axis=1)
        return np.concatenate(
            [wkvp[d * 128:(d + 1) * 128, :] for d in range(NDC)]
            + [wkvm[d * 128:(d + 1) * 128, :] for d in range(NDC)]
            + [wqk[d * 128:(d + 1) * 128, :] for d in range(NDC)]
            + [mask],
            axis=1,
        ).astype(bf16)

    wcats = [make_wcat(0.0), make_wcat(1.0)]
    wotT = Wo.T                                               # [K, D]
    wot2 = np.concatenate([wotT[:, 0:512], wotT[:, 512:1024]],
                          axis=0).astype(bf16)                # [128, 512]
    f8e3 = ml_dtypes.float8_e3m4
    zeros_xp = np.zeros((128, NCHUNK * 1024), dtype=f8e3)
    in_maps = []
    for core in range(8):
        b, h = core // 2, core % 2
        xb = x[b].astype(bf16)                                # [S, D]
        # main: [dd, g*4096 + d*512 + t]
        xm = xb[h * L:(h + 1) * L, :]                         # [2048, 1024]
        xm4 = xm.reshape(NGRP, G, NDC, 128).transpose(3, 0, 2, 1)
        xtm2 = np.ascontiguousarray(xm4.reshape(128, NGRP * 4096))
        # prefix: [dd, ci*1024 + d*128 + t]
        if h:
            xp = x[b][0:L, :].astype(f8e3)
            xp4 = xp.reshape(NCHUNK, C, NDC, 128).transpose(3, 0, 2, 1)
            xtp2 = np.ascontiguousarray(xp4.reshape(128, NCHUNK * 1024))
        else:
            xtp2 = zeros_xp
        m = {
            "xtp2": xtp2,
            "xtm2": xtm2,
            "wcat": wcats[h],
            "wot2": wot2,
            "selc": np.full((C, 1), float(h) / 16.0, np.float32),
        }
        in_maps.append(m)
    return in_maps


def _run(inputs, trace=False):
    from concourse.bass_utils import run_bass_kernel_spmd

    if "nc" not in _cache:
        _cache["nc"] = _build_nc()
    nc = _cache["nc"]
    in_maps = _prep_inputs(
        np.asarray(inputs["x"], np.float32),
        np.asarray(inputs["Wq"], np.float32),
        np.asarray(inputs["Wk"], np.float32),
        np.asarray(inputs["Wv"], np.float32),
        np.asarray(inputs["Wo"], np.float32),
    )
    res = run_bass_kernel_spmd(nc, in_maps, list(range(8)), trace=trace)
    out = np.empty((B, S, D), np.float32)
    for core in range(8):
        b, h = core // 2, core % 2
        o = res.results[core]["out2"].astype(np.float32)
        # [128 t, ci*1024 + dcol] -> [2048, 1024]
        o = o.reshape(128, NCHUNK, 1024).transpose(1, 0, 2).reshape(L, D)
        out[b, h * L:(h + 1) * L, :] = o
    return out, res


def kernel(**inputs) -> np.ndarray:
    out, _ = _run(inputs, trace=False)
    return out
